# revision 22
# baseline (speedup 1.0000x reference)
"""MiMoV2 FlashTopK router kernel for Trainium2 (Bass/Tile), 8-core SPMD.

Reference computation (fp32):
    logits = hidden @ weight.T                      # [T, 256]
    scores = sigmoid(logits)
    sfc    = scores + bias                          # scores_for_choice
    group top-2 sums over 8 groups of 32 -> pick top-4 groups
    mask non-selected groups to -inf, top-8 experts by sfc
    topk_weight = scores[topk_idx] / (sum + 1e-20)

Sharding: tokens split evenly over 8 cores; weight/bias replicated.
The router weight is passed to the NEFF pre-transposed ([H, E]) since it is a
tiny replicated constant and the matmul needs H on partitions.
"""

import numpy as np
from contextlib import ExitStack

import concourse.bass as bass
import concourse.mybir as mybir
import concourse.tile as tile
from concourse import bacc
from concourse.masks import make_identity

F32 = mybir.dt.float32
F32R = mybir.dt.float32r
U16 = mybir.dt.uint16
I32 = mybir.dt.int32

T_TOTAL = 16384
H = 4096
E = 256
K = 8
G = 8            # expert groups
GS = E // G      # group size (32)
N_CORES = 8
T_PC = T_TOTAL // N_CORES   # tokens per core

P = 128          # partitions
ST = 512         # tokens per supertile (matmul moving free dim)
TT = ST // P     # token tiles per supertile (4)
HT = H // P      # hidden tiles (32)

NEG_BIG = -1.0e30


def build_nc(t_pc: int = T_PC, mm_mode: str = "f32", tp_bufs=4, mm_bufs=3,
             lg_bufs=1, hT_bufs=4, split_copy=False, copies="act", st=256,
             repeat=1, acc_pair=False):
    """Build the single-core Bass program (SPMD: same NEFF on all cores).

    mm_mode "f32": exact fp32 matmuls (4 cyc/row).
    mm_mode "f32r": FP32R matmul path (1 cyc/row at N>=256) — inputs to the
    matmul (hidden tiles, transposes, weight) are rounded to FP32R.
    """
    st_ = st
    n_super = t_pc // st_
    assert n_super * st_ == t_pc
    tt = st_ // P
    MDT = F32R if mm_mode == "f32r" else F32

    nc = bacc.Bacc(None, target_bir_lowering=False)

    hidden = nc.dram_tensor("hidden", [t_pc, H], F32, kind="ExternalInput")
    wt = nc.dram_tensor("wt", [H, E], MDT, kind="ExternalInput")  # weight.T
    bias_d = nc.dram_tensor("bias", [1, E], F32, kind="ExternalInput")
    logits_o = nc.dram_tensor("logits", [t_pc, E], F32, kind="ExternalOutput")
    tw_o = nc.dram_tensor("tw", [t_pc, K], F32, kind="ExternalOutput")
    ti_o = nc.dram_tensor("ti", [t_pc, K], I32, kind="ExternalOutput")

    with tile.TileContext(nc) as tc, ExitStack() as ctx:
        consts = ctx.enter_context(tc.tile_pool(name="consts", bufs=1))
        wpool = ctx.enter_context(tc.tile_pool(name="wpool", bufs=1))
        raw_pool = ctx.enter_context(tc.tile_pool(name="raw", bufs=20))
        hT_pool = ctx.enter_context(tc.tile_pool(name="hT", bufs=hT_bufs))
        lgT_pool = ctx.enter_context(tc.tile_pool(name="lgT", bufs=4))
        sco_pool = ctx.enter_context(tc.tile_pool(name="sco", bufs=3))
        small = ctx.enter_context(tc.tile_pool(name="small", bufs=4))
        p_tp = ctx.enter_context(tc.tile_pool(name="ptp", bufs=tp_bufs, space="PSUM"))
        p_mm = ctx.enter_context(tc.tile_pool(name="pmm", bufs=mm_bufs, space="PSUM"))
        p_lg = ctx.enter_context(tc.tile_pool(name="plg", bufs=lg_bufs, space="PSUM"))

        identity = consts.tile([P, P], F32)
        make_identity(nc, identity[:])
        iota_e = consts.tile([P, E], F32)
        nc.gpsimd.iota(
            iota_e[:], pattern=[[1, E]], base=0, channel_multiplier=0,
            allow_small_or_imprecise_dtypes=True,
        )
        bias_row = consts.tile([1, E], F32)
        nc.sync.dma_start(bias_row[:], bias_d[:])
        bias_bc = consts.tile([P, E], F32)
        nc.gpsimd.partition_broadcast(bias_bc[:], bias_row[:])

        # router weight, H on partitions: wT[p, a, e] = weight.T[a*128+p, e]
        wT = wpool.tile([P, HT, E], MDT)
        for a in range(HT):
            nc.scalar.dma_start(wT[:, a, :], wt[a * P:(a + 1) * P, :])

        for s in [x for _ in range(repeat) for x in range(n_super)]:
            # ---- load hidden rows for this supertile (chunked along H
            # so transposes of early h tiles unblock before the full row
            # arrives, and prefetch granularity is fine) ----
            RC = 4              # raw chunks per token tile
            HC = H // RC        # 1024 columns per chunk (8 h tiles)
            raws = []
            for t in range(tt):
                tok0 = (s * tt + t) * P
                chunks = []
                for c in range(RC):
                    ch = raw_pool.tile([P, HC], F32, tag="raw")
                    nc.sync.dma_start(
                        ch[:], hidden[tok0:tok0 + P, c * HC:(c + 1) * HC])
                    chunks.append(ch)
                raws.append(chunks)

            # ---- matmul: logitsT[e, tok] accumulated over h tiles ----
            if acc_pair and st_ <= 256:
                apair = p_mm.tile([P, 2 * st_], F32, tag="acc")
                acc0 = apair[:, 0:st_]
                acc1 = apair[:, st_:2 * st_]
            else:
                acc0t = p_mm.tile([P, st_], F32, tag="acc")
                acc1t = p_mm.tile([P, st_], F32, tag="acc")
                acc0 = acc0t[:]
                acc1 = acc1t[:]
            for h in range(HT):
                tpb = p_tp.tile([P, st_], F32, tag="tp")
                hcol = (h % (HC // P)) * P
                for t in range(tt):
                    nc.tensor.matmul(
                        tpb[:, t * P:(t + 1) * P],
                        raws[t][h // (HC // P)][:, hcol:hcol + P], identity[:],
                        is_transpose=True, start=(t == 0), stop=(t == tt - 1),
                    )
                hTt = hT_pool.tile([P, st_], MDT, tag="hT")
                if split_copy:
                    nc.scalar.copy(hTt[:, 0:st_ // 2], tpb[:, 0:st_ // 2])
                    nc.vector.tensor_copy(hTt[:, st_ // 2:st_], tpb[:, st_ // 2:st_])
                elif copies == "act" or h % 2 == 0:
                    nc.scalar.copy(hTt[:], tpb[:])
                else:
                    nc.vector.tensor_copy(hTt[:], tpb[:])
                if not acc_pair:
                    nc.tensor.matmul(
                        acc0, wT[:, h, 0:P], hTt[:],
                        start=(h == 0), stop=(h == HT - 1),
                    )
                    nc.tensor.matmul(
                        acc1, wT[:, h, P:E], hTt[:],
                        start=(h == 0), stop=(h == HT - 1),
                    )
                else:
                    # both halves share one PSUM bank / one accumulation group
                    nc.tensor.matmul(
                        acc0, wT[:, h, 0:P], hTt[:],
                        start=(h == 0), stop=False,
                    )
                    nc.tensor.matmul(
                        acc1, wT[:, h, P:E], hTt[:],
                        start=False, stop=(h == HT - 1),
                    )

            lgT0 = lgT_pool.tile([P, st_], F32, tag="lgT")
            lgT1 = lgT_pool.tile([P, st_], F32, tag="lgT")
            with tc.high_priority():
                if copies == "act":
                    nc.scalar.copy(lgT0[:], acc0[:])
                    nc.scalar.copy(lgT1[:], acc1[:])
                else:
                    nc.vector.tensor_copy(lgT0[:], acc0[:])
                    nc.vector.tensor_copy(lgT1[:], acc1[:])

            # ---- per token tile: transpose logits back + topk ----
            for t in range(tt):
                tok0 = (s * tt + t) * P
                lgp = p_lg.tile([P, E], F32, tag="lg")
                nc.tensor.matmul(
                    lgp[:, 0:P], lgT0[:, t * P:(t + 1) * P], identity[:],
                    is_transpose=True, start=True, stop=False,
                )
                nc.tensor.matmul(
                    lgp[:, P:E], lgT1[:, t * P:(t + 1) * P], identity[:],
                    is_transpose=True, start=False, stop=True,
                )
                lg_sb = sco_pool.tile([P, E], F32, tag="lg_sb")
                if copies == "act":
                    nc.scalar.copy(lg_sb[:], lgp[:])
                else:
                    nc.vector.tensor_copy(lg_sb[:], lgp[:])
                nc.scalar.dma_start(logits_o[tok0:tok0 + P, :], lg_sb[:])

                scores = sco_pool.tile([P, E], F32, tag="scores")
                nc.scalar.activation(
                    scores[:], lgp[:], func=mybir.ActivationFunctionType.Sigmoid
                )
                sfc = sco_pool.tile([P, E], F32, tag="sfc")
                nc.vector.tensor_add(sfc[:], scores[:], bias_bc[:])

                # per-group top-8 (need top-2); maxes[:, g, :] descending
                maxes = small.tile([P, G, 8], F32, tag="mx8")
                for g in range(G):
                    nc.vector.max(maxes[:, g, :], sfc[:, g * GS:(g + 1) * GS])
                gsc = small.tile([P, G], F32, tag="gsc")
                nc.vector.tensor_add(gsc[:], maxes[:, :, 0], maxes[:, :, 1])

                # 4th largest group score as threshold
                g8 = small.tile([P, 8], F32, tag="g8")
                nc.vector.max(g8[:], gsc[:])
                # big[g] = 0 if selected else NEG_BIG
                big = small.tile([P, G], F32, tag="big")
                nc.vector.tensor_scalar(
                    big[:], gsc[:], g8[:, 3:4], NEG_BIG,
                    op0=mybir.AluOpType.is_lt, op1=mybir.AluOpType.mult,
                )
                tmp = sco_pool.tile([P, G, GS], F32, tag="tmp")
                nc.vector.tensor_add(
                    tmp[:],
                    sfc[:].rearrange("p (g i) -> p g i", g=G),
                    big[:].unsqueeze(-1).to_broadcast([P, G, GS]),
                )

                tmp2 = tmp[:].rearrange("p g i -> p (g i)")
                v8 = small.tile([P, 8], F32, tag="v8")
                i8 = small.tile([P, 8], U16, tag="i8")
                nc.vector.max(v8[:], tmp2)
                nc.vector.max_index(i8[:], v8[:], tmp2)

                i8f = small.tile([P, 8], F32, tag="i8f")
                nc.vector.tensor_copy(i8f[:], i8[:])

                # gather scores at i8 via iota-compare
                w8 = small.tile([P, 8], F32, tag="w8")
                junk = sco_pool.tile([P, E], F32, tag="junk")
                for j in range(K):
                    nc.vector.scalar_tensor_tensor(
                        out=junk[:], in0=iota_e[:], scalar=i8f[:, j:j + 1],
                        in1=scores[:],
                        op0=mybir.AluOpType.is_equal, op1=mybir.AluOpType.mult,
                        accum_out=w8[:, j:j + 1],
                    )

                denom = small.tile([P, 1], F32, tag="den")
                nc.vector.reduce_sum(denom[:], w8[:], axis=mybir.AxisListType.X)
                nc.vector.tensor_scalar_add(denom[:], denom[:], 1e-20)
                rec = small.tile([P, 1], F32, tag="rec")
                nc.vector.reciprocal(rec[:], denom[:])
                twt = small.tile([P, K], F32, tag="twt")
                nc.vector.tensor_scalar_mul(twt[:], w8[:], rec[:])
                tit = small.tile([P, K], I32, tag="tit")
                nc.vector.tensor_copy(tit[:], i8[:])

                nc.scalar.dma_start(tw_o[tok0:tok0 + P, :], twt[:])
                nc.scalar.dma_start(ti_o[tok0:tok0 + P, :], tit[:])

    nc.compile()
    return nc


_NC_CACHE = {}


def _get_nc(t_pc=T_PC, mm_mode="f32"):
    key = (t_pc, mm_mode)
    if key not in _NC_CACHE:
        _NC_CACHE[key] = build_nc(t_pc, mm_mode)
    return _NC_CACHE[key]


def kernel(hidden_states, weight, e_score_correction_bias):
    from concourse.bass_utils import run_bass_kernel_spmd

    hidden_states = np.ascontiguousarray(np.asarray(hidden_states, dtype=np.float32))
    weight = np.asarray(weight, dtype=np.float32)
    bias = np.asarray(e_score_correction_bias, dtype=np.float32)

    wt = np.ascontiguousarray(weight.T)          # [H, E]
    bias2 = np.ascontiguousarray(bias.reshape(1, E))

    nc = _get_nc()
    in_maps = []
    for c in range(N_CORES):
        in_maps.append({
            "hidden": hidden_states[c * T_PC:(c + 1) * T_PC],
            "wt": wt,
            "bias": bias2,
        })
    res = run_bass_kernel_spmd(nc, in_maps, core_ids=list(range(N_CORES)))
    logits = np.concatenate([r["logits"] for r in res.results], axis=0)
    tw = np.concatenate([r["tw"] for r in res.results], axis=0)
    ti = np.concatenate([r["ti"] for r in res.results], axis=0)
    return logits, tw, ti.astype(np.int32)


# revision 24
# speedup vs baseline: 1.0270x; 1.0270x over previous
"""MiMoV2 FlashTopK router kernel for Trainium2 (Bass/Tile), 8-core SPMD.

Reference computation (fp32):
    logits = hidden @ weight.T                      # [T, 256]
    scores = sigmoid(logits)
    sfc    = scores + bias                          # scores_for_choice
    group top-2 sums over 8 groups of 32 -> pick top-4 groups
    mask non-selected groups to -inf, top-8 experts by sfc
    topk_weight = scores[topk_idx] / (sum + 1e-20)

Sharding: tokens split evenly over 8 cores; weight/bias replicated.
The router weight is passed to the NEFF pre-transposed ([H, E]) since it is a
tiny replicated constant and the matmul needs H on partitions.
"""

import numpy as np
from contextlib import ExitStack

import concourse.bass as bass
import concourse.mybir as mybir
import concourse.tile as tile
from concourse import bacc
from concourse.masks import make_identity

F32 = mybir.dt.float32
F32R = mybir.dt.float32r
U16 = mybir.dt.uint16
I32 = mybir.dt.int32

T_TOTAL = 16384
H = 4096
E = 256
K = 8
G = 8            # expert groups
GS = E // G      # group size (32)
N_CORES = 8
T_PC = T_TOTAL // N_CORES   # tokens per core

P = 128          # partitions
ST = 512         # tokens per supertile (matmul moving free dim)
TT = ST // P     # token tiles per supertile (4)
HT = H // P      # hidden tiles (32)

NEG_BIG = -1.0e30


def build_nc(t_pc: int = T_PC, mm_mode: str = "f32", tp_bufs=4, mm_bufs=3,
             lg_bufs=1, hT_bufs=4, split_copy=False, copies="act", st=256,
             repeat=1, acc_pair=False):
    """Build the single-core Bass program (SPMD: same NEFF on all cores).

    mm_mode "f32": exact fp32 matmuls (4 cyc/row).
    mm_mode "f32r": FP32R matmul path (1 cyc/row at N>=256) — inputs to the
    matmul (hidden tiles, transposes, weight) are rounded to FP32R.
    """
    st_ = st
    n_super = t_pc // st_
    assert n_super * st_ == t_pc
    tt = st_ // P
    MDT = F32R if mm_mode == "f32r" else F32

    nc = bacc.Bacc(None, target_bir_lowering=False)

    hidden = nc.dram_tensor("hidden", [t_pc, H], F32, kind="ExternalInput")
    wt = nc.dram_tensor("wt", [H, E], MDT, kind="ExternalInput")  # weight.T
    bias_d = nc.dram_tensor("bias", [1, E], F32, kind="ExternalInput")
    logits_o = nc.dram_tensor("logits", [t_pc, E], F32, kind="ExternalOutput")
    tw_o = nc.dram_tensor("tw", [t_pc, K], F32, kind="ExternalOutput")
    ti_o = nc.dram_tensor("ti", [t_pc, K], I32, kind="ExternalOutput")

    with tile.TileContext(nc) as tc, ExitStack() as ctx:
        consts = ctx.enter_context(tc.tile_pool(name="consts", bufs=1))
        wpool = ctx.enter_context(tc.tile_pool(name="wpool", bufs=1))
        raw_pool = ctx.enter_context(tc.tile_pool(name="raw", bufs=20))
        hT_pool = ctx.enter_context(tc.tile_pool(name="hT", bufs=hT_bufs))
        lgT_pool = ctx.enter_context(tc.tile_pool(name="lgT", bufs=4))
        sco_pool = ctx.enter_context(tc.tile_pool(name="sco", bufs=3))
        small = ctx.enter_context(tc.tile_pool(name="small", bufs=4))
        p_tp = ctx.enter_context(tc.tile_pool(name="ptp", bufs=tp_bufs, space="PSUM"))
        p_mm = ctx.enter_context(tc.tile_pool(name="pmm", bufs=mm_bufs, space="PSUM"))
        p_lg = ctx.enter_context(tc.tile_pool(name="plg", bufs=lg_bufs, space="PSUM"))

        identity = consts.tile([P, P], F32)
        make_identity(nc, identity[:])
        iota_e = consts.tile([P, E], F32)
        nc.gpsimd.iota(
            iota_e[:], pattern=[[1, E]], base=0, channel_multiplier=0,
            allow_small_or_imprecise_dtypes=True,
        )
        bias_row = consts.tile([1, E], F32)
        nc.sync.dma_start(bias_row[:], bias_d[:])
        bias_bc = consts.tile([P, E], F32)
        nc.gpsimd.partition_broadcast(bias_bc[:], bias_row[:])

        # router weight, H on partitions: wT[p, a, e] = weight.T[a*128+p, e].
        # Loaded lazily inside the first supertile's h loop so the early HBM
        # bandwidth goes to the hidden stream instead of the 4MB weight.
        wT = wpool.tile([P, HT, E], MDT)

        for s in [x for _ in range(repeat) for x in range(n_super)]:
            # ---- load hidden rows for this supertile (chunked along H
            # so transposes of early h tiles unblock before the full row
            # arrives, and prefetch granularity is fine) ----
            RC = 4              # raw chunks per token tile
            HC = H // RC        # 1024 columns per chunk (8 h tiles)
            raws = []
            for t in range(tt):
                tok0 = (s * tt + t) * P
                chunks = []
                for c in range(RC):
                    ch = raw_pool.tile([P, HC], F32, tag="raw")
                    nc.sync.dma_start(
                        ch[:], hidden[tok0:tok0 + P, c * HC:(c + 1) * HC])
                    chunks.append(ch)
                raws.append(chunks)

            # ---- matmul: logitsT[e, tok] accumulated over h tiles ----
            if acc_pair and st_ <= 256:
                apair = p_mm.tile([P, 2 * st_], F32, tag="acc")
                acc0 = apair[:, 0:st_]
                acc1 = apair[:, st_:2 * st_]
            else:
                acc0t = p_mm.tile([P, st_], F32, tag="acc")
                acc1t = p_mm.tile([P, st_], F32, tag="acc")
                acc0 = acc0t[:]
                acc1 = acc1t[:]
            for h in range(HT):
                if s == 0:
                    if h < 4:
                        with tc.high_priority():
                            nc.scalar.dma_start(
                                wT[:, h, :], wt[h * P:(h + 1) * P, :])
                    else:
                        nc.scalar.dma_start(
                            wT[:, h, :], wt[h * P:(h + 1) * P, :])
                tpb = p_tp.tile([P, st_], F32, tag="tp")
                hcol = (h % (HC // P)) * P
                for t in range(tt):
                    nc.tensor.matmul(
                        tpb[:, t * P:(t + 1) * P],
                        raws[t][h // (HC // P)][:, hcol:hcol + P], identity[:],
                        is_transpose=True, start=(t == 0), stop=(t == tt - 1),
                    )
                hTt = hT_pool.tile([P, st_], MDT, tag="hT")
                if split_copy:
                    nc.scalar.copy(hTt[:, 0:st_ // 2], tpb[:, 0:st_ // 2])
                    nc.vector.tensor_copy(hTt[:, st_ // 2:st_], tpb[:, st_ // 2:st_])
                elif copies == "act" or h % 2 == 0:
                    nc.scalar.copy(hTt[:], tpb[:])
                else:
                    nc.vector.tensor_copy(hTt[:], tpb[:])
                if not acc_pair:
                    nc.tensor.matmul(
                        acc0, wT[:, h, 0:P], hTt[:],
                        start=(h == 0), stop=(h == HT - 1),
                    )
                    nc.tensor.matmul(
                        acc1, wT[:, h, P:E], hTt[:],
                        start=(h == 0), stop=(h == HT - 1),
                    )
                else:
                    # both halves share one PSUM bank / one accumulation group
                    nc.tensor.matmul(
                        acc0, wT[:, h, 0:P], hTt[:],
                        start=(h == 0), stop=False,
                    )
                    nc.tensor.matmul(
                        acc1, wT[:, h, P:E], hTt[:],
                        start=False, stop=(h == HT - 1),
                    )

            lgT0 = lgT_pool.tile([P, st_], F32, tag="lgT")
            lgT1 = lgT_pool.tile([P, st_], F32, tag="lgT")
            with tc.high_priority():
                if copies == "act":
                    nc.scalar.copy(lgT0[:], acc0[:])
                    nc.scalar.copy(lgT1[:], acc1[:])
                else:
                    nc.vector.tensor_copy(lgT0[:], acc0[:])
                    nc.vector.tensor_copy(lgT1[:], acc1[:])

            # ---- per token tile: transpose logits back + topk ----
            for t in range(tt):
                tok0 = (s * tt + t) * P
                lgp = p_lg.tile([P, E], F32, tag="lg")
                nc.tensor.matmul(
                    lgp[:, 0:P], lgT0[:, t * P:(t + 1) * P], identity[:],
                    is_transpose=True, start=True, stop=False,
                )
                nc.tensor.matmul(
                    lgp[:, P:E], lgT1[:, t * P:(t + 1) * P], identity[:],
                    is_transpose=True, start=False, stop=True,
                )
                lg_sb = sco_pool.tile([P, E], F32, tag="lg_sb")
                if copies == "act":
                    nc.scalar.copy(lg_sb[:], lgp[:])
                else:
                    nc.vector.tensor_copy(lg_sb[:], lgp[:])
                nc.scalar.dma_start(logits_o[tok0:tok0 + P, :], lg_sb[:])

                scores = sco_pool.tile([P, E], F32, tag="scores")
                nc.scalar.activation(
                    scores[:], lgp[:], func=mybir.ActivationFunctionType.Sigmoid
                )
                sfc = sco_pool.tile([P, E], F32, tag="sfc")
                nc.vector.tensor_add(sfc[:], scores[:], bias_bc[:])

                # per-group top-8 (need top-2); maxes[:, g, :] descending
                maxes = small.tile([P, G, 8], F32, tag="mx8")
                for g in range(G):
                    nc.vector.max(maxes[:, g, :], sfc[:, g * GS:(g + 1) * GS])
                gsc = small.tile([P, G], F32, tag="gsc")
                nc.vector.tensor_add(gsc[:], maxes[:, :, 0], maxes[:, :, 1])

                # 4th largest group score as threshold
                g8 = small.tile([P, 8], F32, tag="g8")
                nc.vector.max(g8[:], gsc[:])
                # big[g] = 0 if selected else NEG_BIG
                big = small.tile([P, G], F32, tag="big")
                nc.vector.tensor_scalar(
                    big[:], gsc[:], g8[:, 3:4], NEG_BIG,
                    op0=mybir.AluOpType.is_lt, op1=mybir.AluOpType.mult,
                )
                tmp = sco_pool.tile([P, G, GS], F32, tag="tmp")
                nc.vector.tensor_add(
                    tmp[:],
                    sfc[:].rearrange("p (g i) -> p g i", g=G),
                    big[:].unsqueeze(-1).to_broadcast([P, G, GS]),
                )

                tmp2 = tmp[:].rearrange("p g i -> p (g i)")
                v8 = small.tile([P, 8], F32, tag="v8")
                i8 = small.tile([P, 8], U16, tag="i8")
                nc.vector.max(v8[:], tmp2)
                nc.vector.max_index(i8[:], v8[:], tmp2)

                i8f = small.tile([P, 8], F32, tag="i8f")
                nc.vector.tensor_copy(i8f[:], i8[:])

                # gather scores at i8 via iota-compare
                w8 = small.tile([P, 8], F32, tag="w8")
                junk = sco_pool.tile([P, E], F32, tag="junk")
                for j in range(K):
                    nc.vector.scalar_tensor_tensor(
                        out=junk[:], in0=iota_e[:], scalar=i8f[:, j:j + 1],
                        in1=scores[:],
                        op0=mybir.AluOpType.is_equal, op1=mybir.AluOpType.mult,
                        accum_out=w8[:, j:j + 1],
                    )

                denom = small.tile([P, 1], F32, tag="den")
                nc.vector.reduce_sum(denom[:], w8[:], axis=mybir.AxisListType.X)
                nc.vector.tensor_scalar_add(denom[:], denom[:], 1e-20)
                rec = small.tile([P, 1], F32, tag="rec")
                nc.vector.reciprocal(rec[:], denom[:])
                twt = small.tile([P, K], F32, tag="twt")
                nc.vector.tensor_scalar_mul(twt[:], w8[:], rec[:])
                tit = small.tile([P, K], I32, tag="tit")
                nc.vector.tensor_copy(tit[:], i8[:])

                nc.scalar.dma_start(tw_o[tok0:tok0 + P, :], twt[:])
                nc.scalar.dma_start(ti_o[tok0:tok0 + P, :], tit[:])

    nc.compile()
    return nc


_NC_CACHE = {}


def _get_nc(t_pc=T_PC, mm_mode="f32"):
    key = (t_pc, mm_mode)
    if key not in _NC_CACHE:
        _NC_CACHE[key] = build_nc(t_pc, mm_mode)
    return _NC_CACHE[key]


def kernel(hidden_states, weight, e_score_correction_bias):
    from concourse.bass_utils import run_bass_kernel_spmd

    hidden_states = np.ascontiguousarray(np.asarray(hidden_states, dtype=np.float32))
    weight = np.asarray(weight, dtype=np.float32)
    bias = np.asarray(e_score_correction_bias, dtype=np.float32)

    wt = np.ascontiguousarray(weight.T)          # [H, E]
    bias2 = np.ascontiguousarray(bias.reshape(1, E))

    nc = _get_nc()
    in_maps = []
    for c in range(N_CORES):
        in_maps.append({
            "hidden": hidden_states[c * T_PC:(c + 1) * T_PC],
            "wt": wt,
            "bias": bias2,
        })
    res = run_bass_kernel_spmd(nc, in_maps, core_ids=list(range(N_CORES)))
    logits = np.concatenate([r["logits"] for r in res.results], axis=0)
    tw = np.concatenate([r["tw"] for r in res.results], axis=0)
    ti = np.concatenate([r["ti"] for r in res.results], axis=0)
    return logits, tw, ti.astype(np.int32)


# revision 26
# speedup vs baseline: 1.0286x; 1.0016x over previous
"""MiMoV2 FlashTopK router kernel for Trainium2 (Bass/Tile), 8-core SPMD.

Reference computation (fp32):
    logits = hidden @ weight.T                      # [T, 256]
    scores = sigmoid(logits)
    sfc    = scores + bias                          # scores_for_choice
    group top-2 sums over 8 groups of 32 -> pick top-4 groups
    mask non-selected groups to -inf, top-8 experts by sfc
    topk_weight = scores[topk_idx] / (sum + 1e-20)

Sharding: tokens split evenly over 8 cores; weight/bias replicated.
The router weight is passed to the NEFF pre-transposed ([H, E]) since it is a
tiny replicated constant and the matmul needs H on partitions.
"""

import numpy as np
from contextlib import ExitStack

import concourse.bass as bass
import concourse.mybir as mybir
import concourse.tile as tile
from concourse import bacc
from concourse.masks import make_identity

F32 = mybir.dt.float32
F32R = mybir.dt.float32r
U16 = mybir.dt.uint16
I32 = mybir.dt.int32

T_TOTAL = 16384
H = 4096
E = 256
K = 8
G = 8            # expert groups
GS = E // G      # group size (32)
N_CORES = 8
T_PC = T_TOTAL // N_CORES   # tokens per core

P = 128          # partitions
ST = 512         # tokens per supertile (matmul moving free dim)
TT = ST // P     # token tiles per supertile (4)
HT = H // P      # hidden tiles (32)

NEG_BIG = -1.0e30


def build_nc(t_pc: int = T_PC, mm_mode: str = "f32", tp_bufs=4, mm_bufs=3,
             lg_bufs=1, hT_bufs=6, split_copy=False, copies="act", st=256,
             repeat=1, acc_pair=False, hT_prio=False):
    """Build the single-core Bass program (SPMD: same NEFF on all cores).

    mm_mode "f32": exact fp32 matmuls (4 cyc/row).
    mm_mode "f32r": FP32R matmul path (1 cyc/row at N>=256) — inputs to the
    matmul (hidden tiles, transposes, weight) are rounded to FP32R.
    """
    st_ = st
    n_super = t_pc // st_
    assert n_super * st_ == t_pc
    tt = st_ // P
    MDT = F32R if mm_mode == "f32r" else F32

    nc = bacc.Bacc(None, target_bir_lowering=False)

    hidden = nc.dram_tensor("hidden", [t_pc, H], F32, kind="ExternalInput")
    wt = nc.dram_tensor("wt", [H, E], MDT, kind="ExternalInput")  # weight.T
    bias_d = nc.dram_tensor("bias", [1, E], F32, kind="ExternalInput")
    logits_o = nc.dram_tensor("logits", [t_pc, E], F32, kind="ExternalOutput")
    tw_o = nc.dram_tensor("tw", [t_pc, K], F32, kind="ExternalOutput")
    ti_o = nc.dram_tensor("ti", [t_pc, K], I32, kind="ExternalOutput")

    with tile.TileContext(nc) as tc, ExitStack() as ctx:
        consts = ctx.enter_context(tc.tile_pool(name="consts", bufs=1))
        wpool = ctx.enter_context(tc.tile_pool(name="wpool", bufs=1))
        raw_pool = ctx.enter_context(tc.tile_pool(name="raw", bufs=20))
        hT_pool = ctx.enter_context(tc.tile_pool(name="hT", bufs=hT_bufs))
        lgT_pool = ctx.enter_context(tc.tile_pool(name="lgT", bufs=4))
        sco_pool = ctx.enter_context(tc.tile_pool(name="sco", bufs=3))
        small = ctx.enter_context(tc.tile_pool(name="small", bufs=4))
        p_tp = ctx.enter_context(tc.tile_pool(name="ptp", bufs=tp_bufs, space="PSUM"))
        p_mm = ctx.enter_context(tc.tile_pool(name="pmm", bufs=mm_bufs, space="PSUM"))
        p_lg = ctx.enter_context(tc.tile_pool(name="plg", bufs=lg_bufs, space="PSUM"))

        identity = consts.tile([P, P], F32)
        make_identity(nc, identity[:])
        iota_e = consts.tile([P, E], F32)
        nc.gpsimd.iota(
            iota_e[:], pattern=[[1, E]], base=0, channel_multiplier=0,
            allow_small_or_imprecise_dtypes=True,
        )
        bias_row = consts.tile([1, E], F32)
        nc.sync.dma_start(bias_row[:], bias_d[:])
        bias_bc = consts.tile([P, E], F32)
        nc.gpsimd.partition_broadcast(bias_bc[:], bias_row[:])

        # router weight, H on partitions: wT[p, a, e] = weight.T[a*128+p, e].
        # Loaded lazily inside the first supertile's h loop so the early HBM
        # bandwidth goes to the hidden stream instead of the 4MB weight.
        wT = wpool.tile([P, HT, E], MDT)

        for s in [x for _ in range(repeat) for x in range(n_super)]:
            # ---- load hidden rows for this supertile (chunked along H
            # so transposes of early h tiles unblock before the full row
            # arrives, and prefetch granularity is fine) ----
            RC = 4              # raw chunks per token tile
            HC = H // RC        # 1024 columns per chunk (8 h tiles)
            raws = []
            for t in range(tt):
                tok0 = (s * tt + t) * P
                chunks = []
                for c in range(RC):
                    ch = raw_pool.tile([P, HC], F32, tag="raw")
                    nc.sync.dma_start(
                        ch[:], hidden[tok0:tok0 + P, c * HC:(c + 1) * HC])
                    chunks.append(ch)
                raws.append(chunks)

            # ---- matmul: logitsT[e, tok] accumulated over h tiles ----
            if acc_pair and st_ <= 256:
                apair = p_mm.tile([P, 2 * st_], F32, tag="acc")
                acc0 = apair[:, 0:st_]
                acc1 = apair[:, st_:2 * st_]
            else:
                acc0t = p_mm.tile([P, st_], F32, tag="acc")
                acc1t = p_mm.tile([P, st_], F32, tag="acc")
                acc0 = acc0t[:]
                acc1 = acc1t[:]
            for h in range(HT):
                if s == 0:
                    if h < 4:
                        with tc.high_priority():
                            nc.scalar.dma_start(
                                wT[:, h, :], wt[h * P:(h + 1) * P, :])
                    else:
                        nc.scalar.dma_start(
                            wT[:, h, :], wt[h * P:(h + 1) * P, :])
                tpb = p_tp.tile([P, st_], F32, tag="tp")
                hcol = (h % (HC // P)) * P
                for t in range(tt):
                    nc.tensor.matmul(
                        tpb[:, t * P:(t + 1) * P],
                        raws[t][h // (HC // P)][:, hcol:hcol + P], identity[:],
                        is_transpose=True, start=(t == 0), stop=(t == tt - 1),
                    )
                hTt = hT_pool.tile([P, st_], MDT, tag="hT")
                if hT_prio:
                    with tc.high_priority():
                        nc.scalar.copy(hTt[:], tpb[:])
                elif split_copy:
                    nc.scalar.copy(hTt[:, 0:st_ // 2], tpb[:, 0:st_ // 2])
                    nc.vector.tensor_copy(hTt[:, st_ // 2:st_], tpb[:, st_ // 2:st_])
                elif copies == "act" or h % 2 == 0:
                    nc.scalar.copy(hTt[:], tpb[:])
                else:
                    nc.vector.tensor_copy(hTt[:], tpb[:])
                if not acc_pair:
                    nc.tensor.matmul(
                        acc0, wT[:, h, 0:P], hTt[:],
                        start=(h == 0), stop=(h == HT - 1),
                    )
                    nc.tensor.matmul(
                        acc1, wT[:, h, P:E], hTt[:],
                        start=(h == 0), stop=(h == HT - 1),
                    )
                else:
                    # both halves share one PSUM bank / one accumulation group
                    nc.tensor.matmul(
                        acc0, wT[:, h, 0:P], hTt[:],
                        start=(h == 0), stop=False,
                    )
                    nc.tensor.matmul(
                        acc1, wT[:, h, P:E], hTt[:],
                        start=False, stop=(h == HT - 1),
                    )

            lgT0 = lgT_pool.tile([P, st_], F32, tag="lgT")
            lgT1 = lgT_pool.tile([P, st_], F32, tag="lgT")
            with tc.high_priority():
                if copies == "act":
                    nc.scalar.copy(lgT0[:], acc0[:])
                    nc.scalar.copy(lgT1[:], acc1[:])
                else:
                    nc.vector.tensor_copy(lgT0[:], acc0[:])
                    nc.vector.tensor_copy(lgT1[:], acc1[:])

            # ---- per token tile: transpose logits back + topk ----
            for t in range(tt):
                tok0 = (s * tt + t) * P
                lgp = p_lg.tile([P, E], F32, tag="lg")
                nc.tensor.matmul(
                    lgp[:, 0:P], lgT0[:, t * P:(t + 1) * P], identity[:],
                    is_transpose=True, start=True, stop=False,
                )
                nc.tensor.matmul(
                    lgp[:, P:E], lgT1[:, t * P:(t + 1) * P], identity[:],
                    is_transpose=True, start=False, stop=True,
                )
                lg_sb = sco_pool.tile([P, E], F32, tag="lg_sb")
                if copies == "act":
                    nc.scalar.copy(lg_sb[:], lgp[:])
                else:
                    nc.vector.tensor_copy(lg_sb[:], lgp[:])
                nc.scalar.dma_start(logits_o[tok0:tok0 + P, :], lg_sb[:])

                scores = sco_pool.tile([P, E], F32, tag="scores")
                nc.scalar.activation(
                    scores[:], lgp[:], func=mybir.ActivationFunctionType.Sigmoid
                )
                sfc = sco_pool.tile([P, E], F32, tag="sfc")
                nc.vector.tensor_add(sfc[:], scores[:], bias_bc[:])

                # per-group top-8 (need top-2); maxes[:, g, :] descending
                maxes = small.tile([P, G, 8], F32, tag="mx8")
                for g in range(G):
                    nc.vector.max(maxes[:, g, :], sfc[:, g * GS:(g + 1) * GS])
                gsc = small.tile([P, G], F32, tag="gsc")
                nc.vector.tensor_add(gsc[:], maxes[:, :, 0], maxes[:, :, 1])

                # 4th largest group score as threshold
                g8 = small.tile([P, 8], F32, tag="g8")
                nc.vector.max(g8[:], gsc[:])
                # big[g] = 0 if selected else NEG_BIG
                big = small.tile([P, G], F32, tag="big")
                nc.vector.tensor_scalar(
                    big[:], gsc[:], g8[:, 3:4], NEG_BIG,
                    op0=mybir.AluOpType.is_lt, op1=mybir.AluOpType.mult,
                )
                tmp = sco_pool.tile([P, G, GS], F32, tag="tmp")
                nc.vector.tensor_add(
                    tmp[:],
                    sfc[:].rearrange("p (g i) -> p g i", g=G),
                    big[:].unsqueeze(-1).to_broadcast([P, G, GS]),
                )

                tmp2 = tmp[:].rearrange("p g i -> p (g i)")
                v8 = small.tile([P, 8], F32, tag="v8")
                i8 = small.tile([P, 8], U16, tag="i8")
                nc.vector.max(v8[:], tmp2)
                nc.vector.max_index(i8[:], v8[:], tmp2)

                i8f = small.tile([P, 8], F32, tag="i8f")
                nc.vector.tensor_copy(i8f[:], i8[:])

                # gather scores at i8 via iota-compare
                w8 = small.tile([P, 8], F32, tag="w8")
                junk = sco_pool.tile([P, E], F32, tag="junk")
                for j in range(K):
                    nc.vector.scalar_tensor_tensor(
                        out=junk[:], in0=iota_e[:], scalar=i8f[:, j:j + 1],
                        in1=scores[:],
                        op0=mybir.AluOpType.is_equal, op1=mybir.AluOpType.mult,
                        accum_out=w8[:, j:j + 1],
                    )

                denom = small.tile([P, 1], F32, tag="den")
                nc.vector.reduce_sum(denom[:], w8[:], axis=mybir.AxisListType.X)
                nc.vector.tensor_scalar_add(denom[:], denom[:], 1e-20)
                rec = small.tile([P, 1], F32, tag="rec")
                nc.vector.reciprocal(rec[:], denom[:])
                twt = small.tile([P, K], F32, tag="twt")
                nc.vector.tensor_scalar_mul(twt[:], w8[:], rec[:])
                tit = small.tile([P, K], I32, tag="tit")
                nc.vector.tensor_copy(tit[:], i8[:])

                nc.scalar.dma_start(tw_o[tok0:tok0 + P, :], twt[:])
                nc.scalar.dma_start(ti_o[tok0:tok0 + P, :], tit[:])

    nc.compile()
    return nc


_NC_CACHE = {}


def _get_nc(t_pc=T_PC, mm_mode="f32"):
    key = (t_pc, mm_mode)
    if key not in _NC_CACHE:
        _NC_CACHE[key] = build_nc(t_pc, mm_mode)
    return _NC_CACHE[key]


def kernel(hidden_states, weight, e_score_correction_bias):
    from concourse.bass_utils import run_bass_kernel_spmd

    hidden_states = np.ascontiguousarray(np.asarray(hidden_states, dtype=np.float32))
    weight = np.asarray(weight, dtype=np.float32)
    bias = np.asarray(e_score_correction_bias, dtype=np.float32)

    wt = np.ascontiguousarray(weight.T)          # [H, E]
    bias2 = np.ascontiguousarray(bias.reshape(1, E))

    nc = _get_nc()
    in_maps = []
    for c in range(N_CORES):
        in_maps.append({
            "hidden": hidden_states[c * T_PC:(c + 1) * T_PC],
            "wt": wt,
            "bias": bias2,
        })
    res = run_bass_kernel_spmd(nc, in_maps, core_ids=list(range(N_CORES)))
    logits = np.concatenate([r["logits"] for r in res.results], axis=0)
    tw = np.concatenate([r["tw"] for r in res.results], axis=0)
    ti = np.concatenate([r["ti"] for r in res.results], axis=0)
    return logits, tw, ti.astype(np.int32)
